# revision 1
# baseline (speedup 1.0000x reference)
"""ConvShapeletFilter kernel for Trainium2 (8 NeuronCores, data-parallel).

Math: reference computes, per batch row b and shapelet k,
    corr[b,n,k] = <x_win[b,n]-mean(x_win[b,n]), s[k]-mean(s[k])>
Since (s[k]-mean(s[k])) sums to zero over l, the window-mean term drops:
    corr[b,n,k] = sum_l x[b,n+l] * s_norm[k,l]
i.e. a plain cross-correlation with the mean-removed shapelet bank.
Outputs per (b,k): top-1, mean(top-5), top-2, relu(top1-top2) over n.

Device mapping (per core, 32 of 256 batch rows):
  - s_norm^T prepared host-side (tiny [128,128] op) and shipped as input.
  - Hankel/im2col tiles H[l, w] = x[b, n0+l+w] DMA'd straight from HBM
    with an overlapping access pattern (one ~1MB DMA per row), used as
    the moving operand: corr block = s_norm^T.T @ H -> PSUM fp32.
  - Mode "f32r": fp32 data, matmul ops bitcast to float32r (full-rate
    PE, G=64 filter split with PSUM accumulation to halve DMA traffic).
    Mode "bf16": bf16 data, single G=128 matmul per block.
    Mode "f32": plain fp32 (4 cyc/col PE, G=64 split).
  - DVE InstMax (top-8 per partition) on each half-row PSUM span;
    a tiny second InstMax merges; small ops build the four output
    metrics; one PE transpose + 4 DMAs write y[32, 512].
"""

import os
import sys

for _p in ("/opt/trn_rl_repo", os.path.expanduser("~/.axon_site/_ro/trn_rl_repo")):
    if os.path.isdir(_p) and _p not in sys.path:
        sys.path.insert(0, _p)

import numpy as np

MODE = os.environ.get("SHAPELET_MODE", "f32r")   # f32r | bf16 | f32

B, T = 256, 4096
L = 128
K = 128
K_TOP = 5
N = T - L + 1          # 3969 sliding windows
N_CORES = 8
ROWS = B // N_CORES    # 32 batch rows per core
WBLK = 512             # windows per matmul (1 PSUM bank fp32)
HALF = 2048            # windows per PSUM span (4 banks)
OUT_COLS = 4 * K       # p1 | p_mean | p2 | dominance
G = 64                 # filter-chunk size for f32r/f32 (DMA = G/128 of full)
TPAD = T + 2 * G       # padded x row length for the G-split hankel reads


def _split_excess_waits(nc, mybir, max_waits=1):
    """Walrus CoreV3 codegen rejects >1 sync-wait on several instruction
    classes (CTRL/Drain, S3_LW/Matmult, ...). Hoist excess waits onto
    same-engine NoOps placed just before the offender."""
    for fn in nc.m.functions:
        for bb in fn.blocks:
            insts = bb.instructions
            i = 0
            while i < len(insts):
                inst = insts[i]
                si = inst.sync_info
                if (si is not None and si.on_wait
                        and len(si.on_wait) > max_waits):
                    waits = list(si.on_wait)
                    si.on_wait = waits[:max_waits]
                    for cs in range(max_waits, len(waits), max_waits):
                        chunk = waits[cs:cs + max_waits]
                        d = nc.sync.nop(nofuse=True)
                        cur = nc.cur_bb.bb.instructions
                        assert cur[-1] is d.ins
                        cur.pop()
                        d.ins.engine = inst.engine
                        d.ins.sync_info = mybir.SyncInfo(on_wait=chunk, on_update=[])
                        insts.insert(i, d.ins)
                        i += 1
                i += 1


def build_program(mode=MODE):
    import concourse.bass as bass
    import concourse.mybir as mybir
    from concourse.masks import make_identity
    from concourse.tile import TileContext

    f32 = mybir.dt.float32
    io_dt = {"bf16": mybir.dt.bfloat16,
             "f32r": mybir.dt.float32r,
             "f32": f32}[mode]
    xlen = TPAD
    # bf16: snt is s_norm^T [L, K]. f32/f32r: both G-row chunks of s_norm^T
    # side by side in the free dim, replicated across both partition halves
    # (PE requires lhsT and rhs to share their base partition).
    snt_shape = [L, K] if mode == "bf16" else [128, 2 * K]

    nc = bass.Bass()
    x = nc.declare_dram_parameter("x", [ROWS, xlen], io_dt, isOutput=False)
    snt_in = nc.declare_dram_parameter("snt", snt_shape, io_dt, isOutput=False)
    y = nc.declare_dram_parameter("y", [ROWS, OUT_COLS], f32, isOutput=True)

    def hankel_ap(b, col0, g_rows, width, n_chunk, chunk_step):
        """AP over x: dims (chunk, l, c) -> x[b, col0 + chunk*chunk_step + l + c]."""
        ap = x[b:b + 1, 0:width].copy()
        dims = [[1, g_rows], [1, width]]
        if n_chunk > 1:
            dims = [[chunk_step, n_chunk]] + dims
        ap.ap = mybir.VecI64Pair(dims)
        ap.offset = b * xlen + col0
        return ap

    with TileContext(nc) as tc:
        with (
            tc.tile_pool(name="const", bufs=1) as const_pool,
            tc.tile_pool(name="hank", bufs=3) as hank_pool,
            tc.tile_pool(name="cand", bufs=3) as cand_pool,
            tc.tile_pool(name="rtop", bufs=3) as rtop_pool,
        ):
            snt = const_pool.tile(snt_shape, io_dt)
            nc.sync.dma_start(out=snt[:, :], in_=snt_in[:, :])
            ident = const_pool.tile([128, 128], f32)
            make_identity(nc, ident[:, :])
            # Result accumulator R[k, m*32 + b], m in (p1, p_mean, p2, dom).
            R = const_pool.tile([K, 128], f32)

            halves = [(0, HALF), (HALF, N - HALF)]   # (n0, n_windows)

            with tc.tile_pool(name="psum", bufs=2, space="PSUM") as psum_pool:
                for b in range(ROWS):
                    if mode == "bf16":
                        hts = []
                        for n0, nw in halves:
                            h = hank_pool.tile([L, HALF], io_dt, tag="hank")
                            nc.sync.dma_start(
                                out=h[:, :],
                                in_=hankel_ap(b, n0, L, HALF, 1, 0))
                            hts.append(h)
                    else:
                        # one DMA for both halves: dest rows 0-63 = half A
                        # (cols c -> x[b, c + l']), rows 64-127 = half B
                        # (x[b, HALF + c + l']). Width covers the G-shifted
                        # second matmul pass; x is padded to TPAD host-side.
                        w = HALF + G
                        h = hank_pool.tile([128, w], io_dt, tag="hank")
                        nc.sync.dma_start(
                            out=h[:, :],
                            in_=hankel_ap(b, 0, G, w, 2, HALF))

                    cand = cand_pool.tile([K, 16], f32)
                    for hi, (n0, nw) in enumerate(halves):
                        ps = psum_pool.tile([K, HALF], f32, tag="psum")
                        # uniform 512-wide matmuls (x padded host-side);
                        # windows >= nw are garbage and excluded from the max
                        for j in range(0, HALF, WBLK):
                            if mode == "bf16":
                                nc.tensor.matmul(
                                    ps[:, j:j + WBLK], snt[:, :],
                                    hts[hi][:, j:j + WBLK],
                                    start=True, stop=True)
                            else:
                                r0 = hi * G
                                nc.tensor.matmul(
                                    ps[:, j:j + WBLK],
                                    snt[r0:r0 + G, 0:K],
                                    h[r0:r0 + G, j:j + WBLK],
                                    start=True, stop=False)
                                nc.tensor.matmul(
                                    ps[:, j:j + WBLK],
                                    snt[r0:r0 + G, K:2 * K],
                                    h[r0:r0 + G, j + G:j + G + WBLK],
                                    start=False, stop=True)
                        nc.vector.max(out=cand[:, 8 * hi:8 * (hi + 1)],
                                      in_=ps[:, :nw])

                    rt = rtop_pool.tile([K, 8], f32)
                    nc.vector.max(out=rt[:, :], in_=cand[:, :])
                    # p1, p_mean, p2, dominance -> R cols b, 32+b, 64+b, 96+b.
                    # All finalize ops stay OFF the DVE (the bottleneck):
                    # copies + accumulate-mean on ACT, dominance on GPSIMD.
                    nc.scalar.copy(R[:, b:b + 1], rt[:, 0:1])
                    pm_scratch = rtop_pool.tile([K, K_TOP], f32, tag="pmscr")
                    nc.scalar.activation(pm_scratch[:, :], rt[:, 0:K_TOP],
                                         mybir.ActivationFunctionType.Copy,
                                         scale=1.0 / K_TOP,
                                         accum_out=R[:, 32 + b:33 + b])
                    nc.scalar.copy(R[:, 64 + b:65 + b], rt[:, 1:2])
                    # dominance = relu(p1 - p2); p1 >= p2 always (sorted
                    # max8 output), so the relu is a no-op and a plain
                    # subtract suffices. (GPSIMD rejected by walrus here,
                    # so this tiny op stays on DVE.)
                    nc.vector.tensor_sub(R[:, 96 + b:97 + b], rt[:, 0:1],
                                         rt[:, 1:2])

            # Transpose R -> TR[m*32+b, k]; write y[b, m*128+k].
            with tc.tile_pool(name="tpsum", bufs=1, space="PSUM") as tpsum_pool:
                tr_ps = tpsum_pool.tile([128, 128], f32)
                nc.tensor.transpose(tr_ps[:, :], R[:, :], ident[:, :])
                tr = const_pool.tile([128, 128], f32)
                nc.scalar.copy(tr[:, :], tr_ps[:, :])
                for m in range(4):
                    nc.sync.dma_start(out=y[:, m * K:(m + 1) * K],
                                      in_=tr[m * ROWS:(m + 1) * ROWS, :])

    _split_excess_waits(nc, mybir)
    return nc


_CACHED = {}


def _get_program(mode=MODE):
    if mode not in _CACHED:
        _CACHED[mode] = build_program(mode)
    return _CACHED[mode]


def _prep_inputs(x, shapelets, mode=MODE):
    x = np.ascontiguousarray(x, dtype=np.float32)
    s = np.asarray(shapelets, dtype=np.float32)
    snt = np.ascontiguousarray((s - s.mean(axis=1, keepdims=True)).T)
    x = np.pad(x, ((0, 0), (0, TPAD - T)))
    if mode == "bf16":
        import ml_dtypes
        x = x.astype(ml_dtypes.bfloat16)
        snt = snt.astype(ml_dtypes.bfloat16)
    else:
        # [128, 2K]: G-row chunks side by side, tiled over both halves.
        snt = np.tile(np.concatenate([snt[0:G], snt[G:2 * G]], axis=1), (2, 1))
        snt = np.ascontiguousarray(snt)
    return x, snt


def run_sharded(x, shapelets, mode=MODE, trace=False, **kw):
    from concourse.bass_utils import run_bass_kernel_spmd

    nc = _get_program(mode)
    xp, snt = _prep_inputs(x, shapelets, mode)
    in_maps = [
        {"x": xp[c * ROWS:(c + 1) * ROWS], "snt": snt}
        for c in range(N_CORES)
    ]
    res = run_bass_kernel_spmd(nc, in_maps, list(range(N_CORES)), trace=trace, **kw)
    out = np.concatenate([res.results[c]["y"] for c in range(N_CORES)], axis=0)
    return out, res


def kernel(x, shapelets):
    out, _ = run_sharded(x, shapelets)
    return out



# revision 3
# speedup vs baseline: 4.1662x; 4.1662x over previous
"""ConvShapeletFilter kernel for Trainium2 (8 NeuronCores, data-parallel).

Math: reference computes, per batch row b and shapelet k,
    corr[b,n,k] = <x_win[b,n]-mean(x_win[b,n]), s[k]-mean(s[k])>
Since (s[k]-mean(s[k])) sums to zero over l, the window-mean term drops:
    corr[b,n,k] = sum_l x[b,n+l] * s_norm[k,l]
i.e. a plain cross-correlation with the mean-removed shapelet bank.
Outputs per (b,k): top-1, mean(top-5), top-2, relu(top1-top2) over n.

Device mapping (per core, 32 of 256 batch rows), v2 design:
  - bf16 data path (matmul accumulates fp32 in PSUM; rel-err ~1e-3,
    gate is 2e-2).
  - Full-tap hankel tile per row: H[l, f] = x[b, l + f], [128, 4160]
    bf16, one DMA per row issued on the GPSIMD engine (SWDGE) so the
    128 descriptors spray across all 16 SDMA engines by destination
    partition.  (HWDGE DIRECT2D assigned descriptors by outermost
    source-AP dim -> only 2 of 16 engines carried the im2col traffic
    in the previous version; that DMA serialization was ~95% of the
    kernel span.)
  - s_norm^T [128, 128] is the lone stationary operand (loaded once
    per matmul, never changes): corr block = snt.T @ H -> PSUM fp32.
    4 matmuls of 1024 columns per row, no accumulation splits.
  - DVE InstMax (top-8 per partition) directly on each [K, 2048] /
    [K, 1921] PSUM half-span; tiny merge InstMax; finalize on ACT
    (p1/p2 copies + accumulate-mean) and DVE (dominance subtract).
  - One PE transpose + 4 DMAs write y[32, 512] fp32.
"""

import os
import sys

for _p in ("/opt/trn_rl_repo", os.path.expanduser("~/.axon_site/_ro/trn_rl_repo")):
    if os.path.isdir(_p) and _p not in sys.path:
        sys.path.insert(0, _p)

import numpy as np

B, T = 256, 4096
L = 128
K = 128
K_TOP = 5
N = T - L + 1          # 3969 sliding windows
N_CORES = 8
ROWS = B // N_CORES    # 32 batch rows per core
WBLK = 512             # windows per matmul (PSUM bank = 512 fp32)
HALF = 2048            # windows per PSUM span (4 banks)
OUT_COLS = 4 * K       # p1 | p_mean | p2 | dominance
HW = 4096 + 64         # hankel tile width: f in [0, 4160)
TPAD = L + HW          # padded x row length (last read: 127 + 4159)


def _split_excess_waits(nc, mybir, max_waits=1):
    """Walrus CoreV3 codegen rejects >1 sync-wait on several instruction
    classes (CTRL/Drain, S3_LW/Matmult, ...). Hoist excess waits onto
    same-engine NoOps placed just before the offender."""
    for fn in nc.m.functions:
        for bb in fn.blocks:
            insts = bb.instructions
            i = 0
            while i < len(insts):
                inst = insts[i]
                si = inst.sync_info
                if (si is not None and si.on_wait
                        and len(si.on_wait) > max_waits):
                    waits = list(si.on_wait)
                    si.on_wait = waits[:max_waits]
                    for cs in range(max_waits, len(waits), max_waits):
                        chunk = waits[cs:cs + max_waits]
                        d = nc.sync.nop(nofuse=True)
                        cur = nc.cur_bb.bb.instructions
                        assert cur[-1] is d.ins
                        cur.pop()
                        d.ins.engine = inst.engine
                        d.ins.sync_info = mybir.SyncInfo(on_wait=chunk, on_update=[])
                        insts.insert(i, d.ins)
                        i += 1
                i += 1


def build_program():
    import concourse.bass as bass
    import concourse.mybir as mybir
    from concourse.masks import make_identity
    from concourse.tile import TileContext

    f32 = mybir.dt.float32
    bf16 = mybir.dt.bfloat16

    nc = bass.Bass()
    x = nc.declare_dram_parameter("x", [ROWS, TPAD], bf16, isOutput=False)
    snt_in = nc.declare_dram_parameter("snt", [L, K], bf16, isOutput=False)
    y = nc.declare_dram_parameter("y", [ROWS, OUT_COLS], f32, isOutput=True)

    def hankel_ap(b):
        """AP over x: dims (l, f) -> x[b, l + f]."""
        ap = x[b:b + 1, 0:HW].copy()
        ap.ap = mybir.VecI64Pair([[1, L], [1, HW]])
        ap.offset = b * TPAD
        return ap

    with TileContext(nc) as tc:
        with (
            tc.tile_pool(name="const", bufs=1) as const_pool,
            tc.tile_pool(name="hank", bufs=3) as hank_pool,
            tc.tile_pool(name="cand", bufs=3) as cand_pool,
            tc.tile_pool(name="rtop", bufs=3) as rtop_pool,
        ):
            snt = const_pool.tile([L, K], bf16)
            nc.sync.dma_start(out=snt[:, :], in_=snt_in[:, :])
            ident = const_pool.tile([128, 128], f32)
            make_identity(nc, ident[:, :])
            # Result accumulator R[k, m*32 + b], m in (p1, p_mean, p2, dom).
            R = const_pool.tile([K, 128], f32)

            halves = [(0, HALF), (HALF, N - HALF)]   # (n0, n_windows)

            with tc.tile_pool(name="psum", bufs=2, space="PSUM") as psum_pool:
                for b in range(ROWS):
                    h = hank_pool.tile([L, HW], bf16, tag="hank")
                    nc.gpsimd.dma_start(out=h[:, :], in_=hankel_ap(b))

                    cand = cand_pool.tile([K, 16], f32)
                    for hi, (n0, nw) in enumerate(halves):
                        ps = psum_pool.tile([K, HALF], f32, tag="psum")
                        for j in range(0, HALF, WBLK):
                            nc.tensor.matmul(
                                ps[:, j:j + WBLK], snt[:, :],
                                h[:, n0 + j:n0 + j + WBLK],
                                start=True, stop=True)
                        # windows >= nw are garbage (x zero-padding)
                        nc.vector.max(out=cand[:, 8 * hi:8 * (hi + 1)],
                                      in_=ps[:, :nw])

                    rt = rtop_pool.tile([K, 8], f32)
                    nc.vector.max(out=rt[:, :], in_=cand[:, :])
                    # p1, p_mean, p2, dominance -> R cols b, 32+b, 64+b, 96+b.
                    nc.scalar.copy(R[:, b:b + 1], rt[:, 0:1])
                    pm_scratch = rtop_pool.tile([K, K_TOP], f32, tag="pmscr")
                    nc.scalar.activation(pm_scratch[:, :], rt[:, 0:K_TOP],
                                         mybir.ActivationFunctionType.Copy,
                                         scale=1.0 / K_TOP,
                                         accum_out=R[:, 32 + b:33 + b])
                    nc.scalar.copy(R[:, 64 + b:65 + b], rt[:, 1:2])
                    # dominance = relu(p1 - p2); p1 >= p2 always (sorted
                    # max8 output), so a plain subtract suffices.
                    nc.vector.tensor_sub(R[:, 96 + b:97 + b], rt[:, 0:1],
                                         rt[:, 1:2])

            # Transpose R -> TR[m*32+b, k]; write y[b, m*128+k].
            with tc.tile_pool(name="tpsum", bufs=1, space="PSUM") as tpsum_pool:
                tr_ps = tpsum_pool.tile([128, 128], f32)
                nc.tensor.transpose(tr_ps[:, :], R[:, :], ident[:, :])
                tr = const_pool.tile([128, 128], f32)
                nc.scalar.copy(tr[:, :], tr_ps[:, :])
                for m in range(4):
                    nc.sync.dma_start(out=y[:, m * K:(m + 1) * K],
                                      in_=tr[m * ROWS:(m + 1) * ROWS, :])

    _split_excess_waits(nc, mybir)
    return nc


_CACHED = {}


def _get_program():
    if "v2" not in _CACHED:
        _CACHED["v2"] = build_program()
    return _CACHED["v2"]


def _prep_inputs(x, shapelets):
    import ml_dtypes

    x = np.ascontiguousarray(x, dtype=np.float32)
    s = np.asarray(shapelets, dtype=np.float32)
    snt = np.ascontiguousarray((s - s.mean(axis=1, keepdims=True)).T)
    x = np.pad(x, ((0, 0), (0, TPAD - T)))
    return x.astype(ml_dtypes.bfloat16), snt.astype(ml_dtypes.bfloat16)


def run_sharded(x, shapelets, trace=False, **kw):
    from concourse.bass_utils import run_bass_kernel_spmd

    nc = _get_program()
    xp, snt = _prep_inputs(x, shapelets)
    in_maps = [
        {"x": xp[c * ROWS:(c + 1) * ROWS], "snt": snt}
        for c in range(N_CORES)
    ]
    res = run_bass_kernel_spmd(nc, in_maps, list(range(N_CORES)), trace=trace, **kw)
    out = np.concatenate([res.results[c]["y"] for c in range(N_CORES)], axis=0)
    return out, res


def kernel(x, shapelets):
    out, _ = run_sharded(x, shapelets)
    return out


# revision 9
# speedup vs baseline: 4.1968x; 1.0073x over previous
"""ConvShapeletFilter kernel for Trainium2 (8 NeuronCores, data-parallel).

Math: reference computes, per batch row b and shapelet k,
    corr[b,n,k] = <x_win[b,n]-mean(x_win[b,n]), s[k]-mean(s[k])>
Since (s[k]-mean(s[k])) sums to zero over l, the window-mean term drops:
    corr[b,n,k] = sum_l x[b,n+l] * s_norm[k,l]
i.e. a plain cross-correlation with the mean-removed shapelet bank.
Outputs per (b,k): top-1, mean(top-5), top-2, relu(top1-top2) over n.

Device mapping (per core, 32 of 256 batch rows), v2 design:
  - bf16 data path (matmul accumulates fp32 in PSUM; rel-err ~1e-3,
    gate is 2e-2).
  - Full-tap hankel tile per row: H[l, f] = x[b, l + f], [128, 4160]
    bf16, one DMA per row issued on the GPSIMD engine (SWDGE) so the
    128 descriptors spray across all 16 SDMA engines by destination
    partition.  (HWDGE DIRECT2D assigned descriptors by outermost
    source-AP dim -> only 2 of 16 engines carried the im2col traffic
    in the previous version; that DMA serialization was ~95% of the
    kernel span.)
  - s_norm^T [128, 128] is the lone stationary operand (loaded once
    per matmul, never changes): corr block = snt.T @ H -> PSUM fp32.
    4 matmuls of 1024 columns per row, no accumulation splits.
  - DVE InstMax (top-8 per partition) directly on each [K, 2048] /
    [K, 1921] PSUM half-span; tiny merge InstMax.  (A fold-based
    pre-reduction was tried and reverted: random shapelets give a
    white corr profile, so top-1-per-fold-slot loses the true #2
    whenever #1/#2 sit exactly a fold distance apart — measured
    9e-2 rel err.)  All finalize ops run on ACT: p1/p2 copies,
    accumulate-mean, and dominance as Identity(p2 * -1 + bias=p1).
  - One PE transpose + 4 DMAs write y[32, 512] fp32.
"""

import os
import sys

for _p in ("/opt/trn_rl_repo", os.path.expanduser("~/.axon_site/_ro/trn_rl_repo")):
    if os.path.isdir(_p) and _p not in sys.path:
        sys.path.insert(0, _p)

import numpy as np

B, T = 256, 4096
L = 128
K = 128
K_TOP = 5
N = T - L + 1          # 3969 sliding windows
N_CORES = 8
ROWS = B // N_CORES    # 32 batch rows per core
WBLK = 512             # windows per matmul (PSUM bank = 512 fp32)
HALF = 2048            # windows per PSUM span (4 banks)
OUT_COLS = 4 * K       # p1 | p_mean | p2 | dominance
HW = 4096 + 64         # hankel tile width: f in [0, 4160)
TPAD = L + HW          # padded x row length (last read: 127 + 4159)


def _split_excess_waits(nc, mybir, max_waits=1):
    """Walrus CoreV3 codegen rejects >1 sync-wait on several instruction
    classes (CTRL/Drain, S3_LW/Matmult, ...). Hoist excess waits onto
    same-engine NoOps placed just before the offender."""
    for fn in nc.m.functions:
        for bb in fn.blocks:
            insts = bb.instructions
            i = 0
            while i < len(insts):
                inst = insts[i]
                si = inst.sync_info
                if (si is not None and si.on_wait
                        and len(si.on_wait) > max_waits):
                    waits = list(si.on_wait)
                    si.on_wait = waits[:max_waits]
                    for cs in range(max_waits, len(waits), max_waits):
                        chunk = waits[cs:cs + max_waits]
                        d = nc.sync.nop(nofuse=True)
                        cur = nc.cur_bb.bb.instructions
                        assert cur[-1] is d.ins
                        cur.pop()
                        d.ins.engine = inst.engine
                        d.ins.sync_info = mybir.SyncInfo(on_wait=chunk, on_update=[])
                        insts.insert(i, d.ins)
                        i += 1
                i += 1


def build_program():
    import concourse.bass as bass
    import concourse.mybir as mybir
    from concourse.masks import make_identity
    from concourse.tile import TileContext

    f32 = mybir.dt.float32
    bf16 = mybir.dt.bfloat16

    nc = bass.Bass()
    x = nc.declare_dram_parameter("x", [ROWS, TPAD], bf16, isOutput=False)
    snt_in = nc.declare_dram_parameter("snt", [L, K], bf16, isOutput=False)
    y = nc.declare_dram_parameter("y", [ROWS, OUT_COLS], f32, isOutput=True)

    def hankel_ap(b):
        """AP over x: dims (l, f) -> x[b, l + f]."""
        ap = x[b:b + 1, 0:HW].copy()
        ap.ap = mybir.VecI64Pair([[1, L], [1, HW]])
        ap.offset = b * TPAD
        return ap

    with TileContext(nc) as tc:
        with (
            tc.tile_pool(name="const", bufs=1) as const_pool,
            tc.tile_pool(name="hank", bufs=3) as hank_pool,
            tc.tile_pool(name="cand", bufs=3) as cand_pool,
            tc.tile_pool(name="rtop", bufs=3) as rtop_pool,
        ):
            snt = const_pool.tile([L, K], bf16)
            nc.sync.dma_start(out=snt[:, :], in_=snt_in[:, :])
            ident = const_pool.tile([128, 128], f32)
            make_identity(nc, ident[:, :])
            # Result accumulator R[k, m*32 + b], m in (p1, p_mean, p2, dom).
            R = const_pool.tile([K, 128], f32)

            halves = [(0, HALF), (HALF, N - HALF)]   # (n0, n_windows)

            with tc.tile_pool(name="psum", bufs=2, space="PSUM") as psum_pool:
                for b in range(ROWS):
                    h = hank_pool.tile([L, HW], bf16, tag="hank")
                    nc.gpsimd.dma_start(out=h[:, :], in_=hankel_ap(b))

                    cand = cand_pool.tile([K, 16], f32)
                    for hi, (n0, nw) in enumerate(halves):
                        ps = psum_pool.tile([K, HALF], f32, tag="psum")
                        for j in range(0, HALF, WBLK):
                            w = min(WBLK, nw - j)
                            nc.tensor.matmul(
                                ps[:, j:j + w], snt[:, :],
                                h[:, n0 + j:n0 + j + w],
                                start=True, stop=True)
                        # windows >= nw are garbage (x zero-padding)
                        nc.vector.max(out=cand[:, 8 * hi:8 * (hi + 1)],
                                      in_=ps[:, :nw])

                    rt = rtop_pool.tile([K, 8], f32)
                    nc.vector.max(out=rt[:, :], in_=cand[:, :])
                    # p1, p_mean, p2, dominance -> R cols b, 32+b, 64+b, 96+b.
                    # All finalize ops on ACT; the DVE (bottleneck) only
                    # runs the InstMax ops above.
                    nc.scalar.copy(R[:, b:b + 1], rt[:, 0:1])
                    pm_scratch = rtop_pool.tile([K, K_TOP], f32, tag="pmscr")
                    nc.scalar.activation(pm_scratch[:, :], rt[:, 0:K_TOP],
                                         mybir.ActivationFunctionType.Copy,
                                         scale=1.0 / K_TOP,
                                         accum_out=R[:, 32 + b:33 + b])
                    nc.scalar.copy(R[:, 64 + b:65 + b], rt[:, 1:2])
                    # dominance = relu(p1 - p2) = p1 - p2 (max8 output is
                    # sorted): Identity(p2 * -1 + bias_ap(p1)) on ACT.
                    nc.scalar.activation(R[:, 96 + b:97 + b], rt[:, 1:2],
                                         mybir.ActivationFunctionType.Identity,
                                         bias=rt[:, 0:1], scale=-1.0)

            # Transpose R -> TR[m*32+b, k]; write y[b, m*128+k].
            with tc.tile_pool(name="tpsum", bufs=1, space="PSUM") as tpsum_pool:
                tr_ps = tpsum_pool.tile([128, 128], f32)
                nc.tensor.transpose(tr_ps[:, :], R[:, :], ident[:, :])
                tr = const_pool.tile([128, 128], f32)
                nc.scalar.copy(tr[:, :], tr_ps[:, :])
                for m in range(4):
                    nc.sync.dma_start(out=y[:, m * K:(m + 1) * K],
                                      in_=tr[m * ROWS:(m + 1) * ROWS, :])

    _split_excess_waits(nc, mybir)
    return nc


_CACHED = {}


def _get_program():
    if "v2" not in _CACHED:
        _CACHED["v2"] = build_program()
    return _CACHED["v2"]


def _prep_inputs(x, shapelets):
    import ml_dtypes

    x = np.ascontiguousarray(x, dtype=np.float32)
    s = np.asarray(shapelets, dtype=np.float32)
    snt = np.ascontiguousarray((s - s.mean(axis=1, keepdims=True)).T)
    x = np.pad(x, ((0, 0), (0, TPAD - T)))
    return x.astype(ml_dtypes.bfloat16), snt.astype(ml_dtypes.bfloat16)


def run_sharded(x, shapelets, trace=False, **kw):
    from concourse.bass_utils import run_bass_kernel_spmd

    nc = _get_program()
    xp, snt = _prep_inputs(x, shapelets)
    in_maps = [
        {"x": xp[c * ROWS:(c + 1) * ROWS], "snt": snt}
        for c in range(N_CORES)
    ]
    res = run_bass_kernel_spmd(nc, in_maps, list(range(N_CORES)), trace=trace, **kw)
    out = np.concatenate([res.results[c]["y"] for c in range(N_CORES)], axis=0)
    return out, res


def kernel(x, shapelets):
    out, _ = run_sharded(x, shapelets)
    return out


# revision 11
# speedup vs baseline: 4.2364x; 1.0094x over previous
"""ConvShapeletFilter kernel for Trainium2 (8 NeuronCores, data-parallel).

Math: reference computes, per batch row b and shapelet k,
    corr[b,n,k] = <x_win[b,n]-mean(x_win[b,n]), s[k]-mean(s[k])>
Since (s[k]-mean(s[k])) sums to zero over l, the window-mean term drops:
    corr[b,n,k] = sum_l x[b,n+l] * s_norm[k,l]
i.e. a plain cross-correlation with the mean-removed shapelet bank.
Outputs per (b,k): top-1, mean(top-5), top-2, relu(top1-top2) over n.

Device mapping (per core, 32 of 256 batch rows), v2 design:
  - bf16 data path (matmul accumulates fp32 in PSUM; rel-err ~1e-3,
    gate is 2e-2).
  - Full-tap hankel tile per row: H[l, f] = x[b, l + f], [128, 4160]
    bf16, one DMA per row issued on the GPSIMD engine (SWDGE) so the
    128 descriptors spray across all 16 SDMA engines by destination
    partition.  (HWDGE DIRECT2D assigned descriptors by outermost
    source-AP dim -> only 2 of 16 engines carried the im2col traffic
    in the previous version; that DMA serialization was ~95% of the
    kernel span.)
  - s_norm^T [128, 128] is the lone stationary operand (loaded once
    per matmul, never changes): corr block = snt.T @ H -> PSUM fp32.
    4 matmuls of 1024 columns per row, no accumulation splits.
  - DVE InstMax (top-8 per partition) directly on each [K, 2048] /
    [K, 1921] PSUM half-span; tiny merge InstMax.  (A fold-based
    pre-reduction was tried and reverted: random shapelets give a
    white corr profile, so top-1-per-fold-slot loses the true #2
    whenever #1/#2 sit exactly a fold distance apart — measured
    9e-2 rel err.)  All finalize ops run on ACT: p1/p2 copies,
    accumulate-mean, and dominance as Identity(p2 * -1 + bias=p1).
  - One PE transpose + 4 DMAs write y[32, 512] fp32.
"""

import os
import sys

for _p in ("/opt/trn_rl_repo", os.path.expanduser("~/.axon_site/_ro/trn_rl_repo")):
    if os.path.isdir(_p) and _p not in sys.path:
        sys.path.insert(0, _p)

import numpy as np

B, T = 256, 4096
L = 128
K = 128
K_TOP = 5
N = T - L + 1          # 3969 sliding windows
N_CORES = 8
ROWS = B // N_CORES    # 32 batch rows per core
WBLK = 512             # windows per matmul (PSUM bank = 512 fp32)
HALF = 2048            # windows per PSUM span (4 banks)
OUT_COLS = 4 * K       # p1 | p_mean | p2 | dominance
HW = 4096 + 64         # hankel tile width: f in [0, 4160)
TPAD = L + HW          # padded x row length (last read: 127 + 4159)


def _split_excess_waits(nc, mybir, max_waits=1):
    """Walrus CoreV3 codegen rejects >1 sync-wait on several instruction
    classes (CTRL/Drain, S3_LW/Matmult, ...). Hoist excess waits onto
    same-engine NoOps placed just before the offender."""
    for fn in nc.m.functions:
        for bb in fn.blocks:
            insts = bb.instructions
            i = 0
            while i < len(insts):
                inst = insts[i]
                si = inst.sync_info
                if (si is not None and si.on_wait
                        and len(si.on_wait) > max_waits):
                    waits = list(si.on_wait)
                    si.on_wait = waits[:max_waits]
                    for cs in range(max_waits, len(waits), max_waits):
                        chunk = waits[cs:cs + max_waits]
                        d = nc.sync.nop(nofuse=True)
                        cur = nc.cur_bb.bb.instructions
                        assert cur[-1] is d.ins
                        cur.pop()
                        d.ins.engine = inst.engine
                        d.ins.sync_info = mybir.SyncInfo(on_wait=chunk, on_update=[])
                        insts.insert(i, d.ins)
                        i += 1
                i += 1


def build_program():
    import concourse.bass as bass
    import concourse.mybir as mybir
    from concourse.masks import make_identity
    from concourse.tile import TileContext

    f32 = mybir.dt.float32
    bf16 = mybir.dt.bfloat16

    nc = bass.Bass()
    x = nc.declare_dram_parameter("x", [ROWS, TPAD], bf16, isOutput=False)
    snt_in = nc.declare_dram_parameter("snt", [L, K], bf16, isOutput=False)
    y = nc.declare_dram_parameter("y", [ROWS, OUT_COLS], f32, isOutput=True)

    def hankel_ap(b):
        """AP over x: dims (l, f) -> x[b, l + f]."""
        ap = x[b:b + 1, 0:HW].copy()
        ap.ap = mybir.VecI64Pair([[1, L], [1, HW]])
        ap.offset = b * TPAD
        return ap

    with TileContext(nc) as tc:
        with (
            tc.tile_pool(name="const", bufs=1) as const_pool,
            tc.tile_pool(name="hank", bufs=3) as hank_pool,
            tc.tile_pool(name="cand", bufs=3) as cand_pool,
            tc.tile_pool(name="rtop", bufs=3) as rtop_pool,
        ):
            snt = const_pool.tile([L, K], bf16)
            nc.sync.dma_start(out=snt[:, :], in_=snt_in[:, :])
            ident = const_pool.tile([128, 128], f32)
            make_identity(nc, ident[:, :])
            # Result accumulator R[k, m*32 + b], m in (p1, p_mean, p2, dom).
            R = const_pool.tile([K, 128], f32)

            # (n0, n_windows) spans; a span lives in one PSUM tile and
            # gets one InstMax.  Row 0 leads with a short span so the
            # DVE starts ~4us earlier (pipeline fill); row 31 ends with
            # a short span to shorten the drain into the output tail.
            spans_std = [(0, HALF), (HALF, N - HALF)]
            spans_first = [(0, WBLK), (WBLK, HALF - WBLK), (HALF, N - HALF)]
            spans_last = [(0, HALF), (HALF, N - HALF - WBLK),
                          (N - WBLK, WBLK)]

            with tc.tile_pool(name="psum", bufs=2, space="PSUM") as psum_pool:
                for b in range(ROWS):
                    h = hank_pool.tile([L, HW], bf16, tag="hank")
                    nc.gpsimd.dma_start(out=h[:, :], in_=hankel_ap(b))

                    spans = (spans_first if b == 0
                             else spans_last if b == ROWS - 1
                             else spans_std)
                    cand = cand_pool.tile([K, 8 * len(spans)], f32,
                                          tag="cand")
                    for hi, (n0, nw) in enumerate(spans):
                        ps = psum_pool.tile([K, HALF], f32, tag="psum")
                        for j in range(0, nw, WBLK):
                            w = min(WBLK, nw - j)
                            nc.tensor.matmul(
                                ps[:, j:j + w], snt[:, :],
                                h[:, n0 + j:n0 + j + w],
                                start=True, stop=True)
                        # windows >= nw are garbage (x zero-padding)
                        nc.vector.max(out=cand[:, 8 * hi:8 * (hi + 1)],
                                      in_=ps[:, :nw])

                    rt = rtop_pool.tile([K, 8], f32)
                    nc.vector.max(out=rt[:, :], in_=cand[:, :])
                    # p1, p_mean, p2, dominance -> R cols b, 32+b, 64+b, 96+b.
                    # All finalize ops on ACT; the DVE (bottleneck) only
                    # runs the InstMax ops above.
                    nc.scalar.copy(R[:, b:b + 1], rt[:, 0:1])
                    pm_scratch = rtop_pool.tile([K, K_TOP], f32, tag="pmscr")
                    nc.scalar.activation(pm_scratch[:, :], rt[:, 0:K_TOP],
                                         mybir.ActivationFunctionType.Copy,
                                         scale=1.0 / K_TOP,
                                         accum_out=R[:, 32 + b:33 + b])
                    nc.scalar.copy(R[:, 64 + b:65 + b], rt[:, 1:2])
                    # dominance = relu(p1 - p2) = p1 - p2 (max8 output is
                    # sorted): Identity(p2 * -1 + bias_ap(p1)) on ACT.
                    nc.scalar.activation(R[:, 96 + b:97 + b], rt[:, 1:2],
                                         mybir.ActivationFunctionType.Identity,
                                         bias=rt[:, 0:1], scale=-1.0)

            # Transpose R -> TR[m*32+b, k]; write y[b, m*128+k].
            with tc.tile_pool(name="tpsum", bufs=1, space="PSUM") as tpsum_pool:
                tr_ps = tpsum_pool.tile([128, 128], f32)
                nc.tensor.transpose(tr_ps[:, :], R[:, :], ident[:, :])
                tr = const_pool.tile([128, 128], f32)
                nc.scalar.copy(tr[:, :], tr_ps[:, :])
                for m in range(4):
                    nc.sync.dma_start(out=y[:, m * K:(m + 1) * K],
                                      in_=tr[m * ROWS:(m + 1) * ROWS, :])

    _split_excess_waits(nc, mybir)
    return nc


_CACHED = {}


def _get_program():
    if "v2" not in _CACHED:
        _CACHED["v2"] = build_program()
    return _CACHED["v2"]


def _prep_inputs(x, shapelets):
    import ml_dtypes

    x = np.ascontiguousarray(x, dtype=np.float32)
    s = np.asarray(shapelets, dtype=np.float32)
    snt = np.ascontiguousarray((s - s.mean(axis=1, keepdims=True)).T)
    x = np.pad(x, ((0, 0), (0, TPAD - T)))
    return x.astype(ml_dtypes.bfloat16), snt.astype(ml_dtypes.bfloat16)


def run_sharded(x, shapelets, trace=False, **kw):
    from concourse.bass_utils import run_bass_kernel_spmd

    nc = _get_program()
    xp, snt = _prep_inputs(x, shapelets)
    in_maps = [
        {"x": xp[c * ROWS:(c + 1) * ROWS], "snt": snt}
        for c in range(N_CORES)
    ]
    res = run_bass_kernel_spmd(nc, in_maps, list(range(N_CORES)), trace=trace, **kw)
    out = np.concatenate([res.results[c]["y"] for c in range(N_CORES)], axis=0)
    return out, res


def kernel(x, shapelets):
    out, _ = run_sharded(x, shapelets)
    return out


# revision 13
# speedup vs baseline: 4.2657x; 1.0069x over previous
"""ConvShapeletFilter kernel for Trainium2 (8 NeuronCores, data-parallel).

Math: reference computes, per batch row b and shapelet k,
    corr[b,n,k] = <x_win[b,n]-mean(x_win[b,n]), s[k]-mean(s[k])>
Since (s[k]-mean(s[k])) sums to zero over l, the window-mean term drops:
    corr[b,n,k] = sum_l x[b,n+l] * s_norm[k,l]
i.e. a plain cross-correlation with the mean-removed shapelet bank.
Outputs per (b,k): top-1, mean(top-5), top-2, relu(top1-top2) over n.

Device mapping (per core, 32 of 256 batch rows), v2 design:
  - bf16 data path (matmul accumulates fp32 in PSUM; rel-err ~1e-3,
    gate is 2e-2).
  - Full-tap hankel tile per row: H[l, f] = x[b, l + f], [128, 4160]
    bf16, one DMA per row issued on the GPSIMD engine (SWDGE) so the
    128 descriptors spray across all 16 SDMA engines by destination
    partition.  (HWDGE DIRECT2D assigned descriptors by outermost
    source-AP dim -> only 2 of 16 engines carried the im2col traffic
    in the previous version; that DMA serialization was ~95% of the
    kernel span.)
  - s_norm^T [128, 128] is the lone stationary operand (loaded once
    per matmul, never changes): corr block = snt.T @ H -> PSUM fp32.
    4 matmuls of 1024 columns per row, no accumulation splits.
  - DVE InstMax (top-8 per partition) directly on each [K, 2048] /
    [K, 1921] PSUM half-span; tiny merge InstMax.  (A fold-based
    pre-reduction was tried and reverted: random shapelets give a
    white corr profile, so top-1-per-fold-slot loses the true #2
    whenever #1/#2 sit exactly a fold distance apart — measured
    9e-2 rel err.)  All finalize ops run on ACT: p1/p2 copies,
    accumulate-mean, and dominance as Identity(p2 * -1 + bias=p1).
  - One PE transpose + 4 DMAs write y[32, 512] fp32.
"""

import os
import sys

for _p in ("/opt/trn_rl_repo", os.path.expanduser("~/.axon_site/_ro/trn_rl_repo")):
    if os.path.isdir(_p) and _p not in sys.path:
        sys.path.insert(0, _p)

import numpy as np

B, T = 256, 4096
L = 128
K = 128
K_TOP = 5
N = T - L + 1          # 3969 sliding windows
N_CORES = 8
ROWS = B // N_CORES    # 32 batch rows per core
WBLK = 512             # windows per matmul (PSUM bank = 512 fp32)
HALF = 2048            # windows per PSUM span (4 banks)
OUT_COLS = 4 * K       # p1 | p_mean | p2 | dominance
HW = 4096 + 64         # hankel tile width: f in [0, 4160)
TPAD = L + HW          # padded x row length (last read: 127 + 4159)


def _split_excess_waits(nc, mybir, max_waits=1):
    """Walrus CoreV3 codegen rejects >1 sync-wait on several instruction
    classes (CTRL/Drain, S3_LW/Matmult, ...). Hoist excess waits onto
    same-engine NoOps placed just before the offender."""
    for fn in nc.m.functions:
        for bb in fn.blocks:
            insts = bb.instructions
            i = 0
            while i < len(insts):
                inst = insts[i]
                si = inst.sync_info
                if (si is not None and si.on_wait
                        and len(si.on_wait) > max_waits):
                    waits = list(si.on_wait)
                    si.on_wait = waits[:max_waits]
                    for cs in range(max_waits, len(waits), max_waits):
                        chunk = waits[cs:cs + max_waits]
                        d = nc.sync.nop(nofuse=True)
                        cur = nc.cur_bb.bb.instructions
                        assert cur[-1] is d.ins
                        cur.pop()
                        d.ins.engine = inst.engine
                        d.ins.sync_info = mybir.SyncInfo(on_wait=chunk, on_update=[])
                        insts.insert(i, d.ins)
                        i += 1
                i += 1


def build_program():
    import concourse.bass as bass
    import concourse.mybir as mybir
    from concourse.masks import make_identity
    from concourse.tile import TileContext

    f32 = mybir.dt.float32
    bf16 = mybir.dt.bfloat16

    nc = bass.Bass()
    x = nc.declare_dram_parameter("x", [ROWS, TPAD], bf16, isOutput=False)
    snt_in = nc.declare_dram_parameter("snt", [L, K], bf16, isOutput=False)
    y = nc.declare_dram_parameter("y", [ROWS, OUT_COLS], f32, isOutput=True)

    def hankel_ap(b):
        """AP over x: dims (l, f) -> x[b, l + f]."""
        ap = x[b:b + 1, 0:HW].copy()
        ap.ap = mybir.VecI64Pair([[1, L], [1, HW]])
        ap.offset = b * TPAD
        return ap

    with TileContext(nc) as tc:
        with (
            tc.tile_pool(name="const", bufs=1) as const_pool,
            tc.tile_pool(name="hank", bufs=3) as hank_pool,
            tc.tile_pool(name="cand", bufs=3) as cand_pool,
            tc.tile_pool(name="rtop", bufs=3) as rtop_pool,
        ):
            snt = const_pool.tile([L, K], bf16)
            nc.sync.dma_start(out=snt[:, :], in_=snt_in[:, :])
            ident = const_pool.tile([128, 128], f32)
            make_identity(nc, ident[:, :])
            # Result accumulator R[k, m*32 + b], m in (p1, p_mean, p2, dom).
            R = const_pool.tile([K, 128], f32)

            # (n0, n_windows) spans; a span lives in one PSUM tile and
            # gets one InstMax.  Row 0 leads with a short span so the
            # DVE starts ~4us earlier (pipeline fill); row 31 ends with
            # a short span to shorten the drain into the output tail.
            spans_std = [(0, HALF), (HALF, N - HALF)]
            spans_first = [(0, WBLK), (WBLK, HALF - WBLK), (HALF, N - HALF)]
            spans_last = [(0, HALF), (HALF, N - HALF - WBLK),
                          (N - WBLK, WBLK)]

            with tc.tile_pool(name="psum", bufs=2, space="PSUM") as psum_pool:
                for b in range(ROWS):
                    h = hank_pool.tile([L, HW], bf16, tag="hank")
                    if b == 0:
                        # split row 0's load so the first matmul (and the
                        # DVE behind it) starts ~2.5us earlier
                        ap = hankel_ap(b)
                        ap0 = ap.copy()
                        ap0.ap = mybir.VecI64Pair([[1, L], [1, WBLK]])
                        nc.gpsimd.dma_start(out=h[:, 0:WBLK], in_=ap0)
                        ap1 = ap.copy()
                        ap1.ap = mybir.VecI64Pair([[1, L], [1, HW - WBLK]])
                        ap1.offset = ap.offset + WBLK
                        nc.gpsimd.dma_start(out=h[:, WBLK:HW], in_=ap1)
                    else:
                        nc.gpsimd.dma_start(out=h[:, :], in_=hankel_ap(b))

                    spans = (spans_first if b == 0
                             else spans_last if b == ROWS - 1
                             else spans_std)
                    cand = cand_pool.tile([K, 8 * len(spans)], f32,
                                          tag="cand")
                    for hi, (n0, nw) in enumerate(spans):
                        ps = psum_pool.tile([K, HALF], f32, tag="psum")
                        for j in range(0, nw, WBLK):
                            w = min(WBLK, nw - j)
                            nc.tensor.matmul(
                                ps[:, j:j + w], snt[:, :],
                                h[:, n0 + j:n0 + j + w],
                                start=True, stop=True)
                        # windows >= nw are garbage (x zero-padding)
                        nc.vector.max(out=cand[:, 8 * hi:8 * (hi + 1)],
                                      in_=ps[:, :nw])

                    rt = rtop_pool.tile([K, 8], f32)
                    nc.vector.max(out=rt[:, :], in_=cand[:, :])
                    # p1, p_mean, p2, dominance -> R cols b, 32+b, 64+b, 96+b.
                    # All finalize ops on ACT; the DVE (bottleneck) only
                    # runs the InstMax ops above.
                    nc.scalar.copy(R[:, b:b + 1], rt[:, 0:1])
                    pm_scratch = rtop_pool.tile([K, K_TOP], f32, tag="pmscr")
                    nc.scalar.activation(pm_scratch[:, :], rt[:, 0:K_TOP],
                                         mybir.ActivationFunctionType.Copy,
                                         scale=1.0 / K_TOP,
                                         accum_out=R[:, 32 + b:33 + b])
                    nc.scalar.copy(R[:, 64 + b:65 + b], rt[:, 1:2])
                    # dominance = relu(p1 - p2) = p1 - p2 (max8 output is
                    # sorted): Identity(p2 * -1 + bias_ap(p1)) on ACT.
                    nc.scalar.activation(R[:, 96 + b:97 + b], rt[:, 1:2],
                                         mybir.ActivationFunctionType.Identity,
                                         bias=rt[:, 0:1], scale=-1.0)

            # Transpose R -> TR[m*32+b, k]; write y[b, m*128+k].
            with tc.tile_pool(name="tpsum", bufs=1, space="PSUM") as tpsum_pool:
                tr_ps = tpsum_pool.tile([128, 128], f32)
                nc.tensor.transpose(tr_ps[:, :], R[:, :], ident[:, :])
                tr = const_pool.tile([128, 128], f32)
                nc.scalar.copy(tr[:, :], tr_ps[:, :])
                # split output stores across both HWDGE queues (sync +
                # scalar) to overlap their first-byte latencies
                for m in range(4):
                    eng = nc.sync if m % 2 == 0 else nc.scalar
                    eng.dma_start(out=y[:, m * K:(m + 1) * K],
                                  in_=tr[m * ROWS:(m + 1) * ROWS, :])

    _split_excess_waits(nc, mybir)
    return nc


_CACHED = {}


def _get_program():
    if "v2" not in _CACHED:
        _CACHED["v2"] = build_program()
    return _CACHED["v2"]


def _prep_inputs(x, shapelets):
    import ml_dtypes

    x = np.ascontiguousarray(x, dtype=np.float32)
    s = np.asarray(shapelets, dtype=np.float32)
    snt = np.ascontiguousarray((s - s.mean(axis=1, keepdims=True)).T)
    x = np.pad(x, ((0, 0), (0, TPAD - T)))
    return x.astype(ml_dtypes.bfloat16), snt.astype(ml_dtypes.bfloat16)


def run_sharded(x, shapelets, trace=False, **kw):
    from concourse.bass_utils import run_bass_kernel_spmd

    nc = _get_program()
    xp, snt = _prep_inputs(x, shapelets)
    in_maps = [
        {"x": xp[c * ROWS:(c + 1) * ROWS], "snt": snt}
        for c in range(N_CORES)
    ]
    res = run_bass_kernel_spmd(nc, in_maps, list(range(N_CORES)), trace=trace, **kw)
    out = np.concatenate([res.results[c]["y"] for c in range(N_CORES)], axis=0)
    return out, res


def kernel(x, shapelets):
    out, _ = run_sharded(x, shapelets)
    return out
